# revision 1
# baseline (speedup 1.0000x reference)
"""Trainium2 Bass kernel for nn_CrossAttentionBlock — all-fp8 pipeline.

Reference computation (per batch b of 16):
    q  = einsum('chw,cp->hwp', x[b], Wq)      # (HW=4096, P=512)
    kt = einsum('nd,dp->pn',  y[b], Wk)       # (P, N=128)
    v  = y[b] @ Wv                            # (N, P)
    s  = (q @ kt) / sqrt(C)                   # (HW, N)
    a  = softmax(s, axis=HW)                  # over the SPATIAL axis
    o  = (a @ v) @ Wout                       # (HW, C)
    out = x + o.T.reshape(C, H, W)

Sharding: pure data-parallel over batch, 2 batches per core, no
collectives.

Key observations exploited here:
  * N=128 << HW so the Q/K projections fold into per-batch matrices:
        M = (Wq/sqrt(C)) @ (Wk^T y^T)   (C, N)   once per batch
        s^T = M^T x                      (N, HW)
        VW  = (v^T)^T Wout / Z           (N, C)   [softmax norm folded]
        attnout^T = VW^T a^T             (C, HW)
  * attnout is TINY relative to x (sigma ~0.005 vs 1), and the final
    result is x + attnout, so the whole attention path tolerates fp8
    easily when the residual add happens in fp32 ON THE HOST.  The
    device reads x as fp8(x*2^3), computes everything in fp8 with
    power-of-two scale management, and ships fp8(attnout*2^9) back.
    Measured end-to-end rel err ~4e-3 (gate 2e-2).
  * fp8 halves/quarters DMA (the roofline here) and enables DoubleRow
    matmuls (2 k-subtiles per instruction at 0.5 cycles/row).

Scale bookkeeping (all powers of two; softmax is invariant to the
exp-output scale since Z comes from the same activation's accumulator):
    x' = x*2^3   y' = y*2^3   Wk' = Wk*2^7   Wv' = Wv*2^7
    Wq' = (Wq/sqrt(C))*2^12   Wo' = Wout*2^6
    kt_psum = Wk'^T y'     = kt*2^10 ; kt8 = kt_psum*2^-7
    M_psum  = Wq'^T kt8    = M *2^15 ; m8  = M_psum *2^-8
    s_psum  = m8^T x'      = s *2^10 ; a8  = exp(s_psum*2^-10 - ln4)
    vt_psum = Wv'^T y'     = vT*2^10 ; vt8 = vt_psum*2^-7
    VW_psum = vt8^T Wo'    = VWraw*2^9 ; vw8 = VW_psum * (1/Z')
    out_psum = vw8^T a8    = attnout*2^9 ; host adds x + out8*2^-9
"""

import sys

sys.path.insert(0, "/opt/trn_rl_repo")

import numpy as np
import ml_dtypes

import concourse.bass as bass
import concourse.mybir as mybir
import concourse.tile as tile
from concourse.vector_clock import ScopedClock

B, C, H, W = 16, 512, 64, 64
HW = H * W
N_COND, D_COND, P = 128, 1024, 512
N_CORES = 8
BPC = B // N_CORES  # batches per core

F32 = mybir.dt.float32
F8 = mybir.dt.float8e4
U8 = mybir.dt.uint8
NPF8 = ml_dtypes.float8_e4m3
AX = mybir.AxisListType.X
EXP = mybir.ActivationFunctionType.Exp
DR = mybir.MatmulPerfMode.DoubleRow

PC = C // 128       # 4 chunks over C
PP = P // 128       # 4 chunks over P
PD = D_COND // 128  # 8 chunks over D
NJJ = HW // 1024    # 4 hw super-chunks of 1024
NB = BPC * 128      # 256: batch-concatenated N columns

LN4 = float(np.log(4.0))


class SplitDrainTileContext(tile.TileContext):
    """This walrus build accepts only one sem wait per CTRL/drain
    instruction; Tile's tail drain waits on the whole global clock.
    Split the waits across a chain of drains on SP."""

    MAX_WAITS = 1

    def _drain_and_barrier(self, tick_clock, wait_clock):
        drain_inst = self.nc.sync.drain()
        wait_clock.add_sem_waits(
            drain_inst.ins, ScopedClock({None: tick_clock.global_clock})
        )
        si = drain_inst.ins.sync_info
        if si is not None and si.on_wait and len(si.on_wait) > self.MAX_WAITS:
            waits = list(si.on_wait)
            drain_inst.ins.sync_info = mybir.SyncInfo(
                on_wait=waits[: self.MAX_WAITS],
                on_update=list(si.on_update or []),
            )
            for i in range(self.MAX_WAITS, len(waits), self.MAX_WAITS):
                extra = self.nc.sync.drain()
                extra.ins.sync_info = mybir.SyncInfo(
                    on_wait=waits[i : i + self.MAX_WAITS], on_update=[]
                )
        self.nc.all_engine_barrier()
        assert self.sems is not None
        popped = self.nc._tile_sem_poison_stack.pop()
        assert popped is self._sem_poison
        self.nc.clear_and_free_semaphores(list(self.sems.allocated().values()))
        self.nc.all_engine_barrier()


def split_multi_waits(nc):
    """This walrus build's codegen accepts at most ONE sem wait per
    instruction (any struct type). Split extra waits onto same-engine
    NoOps inserted immediately before the instruction."""
    ctr = [0]
    for fn in nc.m.functions:
        for bb in fn.blocks:
            insts = bb.instructions
            new = []
            changed = False
            for inst in insts:
                si = inst.sync_info
                if si is not None and si.on_wait and len(si.on_wait) > 1:
                    waits = list(si.on_wait)
                    for w in waits[:-1]:
                        nop = mybir.InstNoOp(
                            name=f"I-wsplit-{ctr[0]}", ins=[], outs=[]
                        )
                        ctr[0] += 1
                        nop.engine = inst.engine
                        nop.sync_info = mybir.SyncInfo(on_wait=[w], on_update=[])
                        new.append(nop)
                    inst.sync_info = mybir.SyncInfo(
                        on_wait=[waits[-1]], on_update=list(si.on_update or [])
                    )
                    changed = True
                new.append(inst)
            if changed:
                bb.instructions = new


def u8(ap):
    """View an fp8 AP as uint8 so DMA endpoints match the u8 DRAM
    tensors (avoids fp8 dtypes in the jax/PJRT boundary)."""
    return ap.bitcast(U8)


def build_nc(reps: int = 1, split_waits: bool = True) -> bass.Bass:
    nc = bass.Bass()

    # All DRAM tensors are uint8 views of fp8 payloads, host-packed so
    # every load is one contiguous-per-partition DMA.
    xq = nc.declare_dram_parameter("xq", [BPC, 128, PC, HW], U8, isOutput=False)
    yq = nc.declare_dram_parameter("yq", [128, PD, BPC, 128], U8, isOutput=False)
    wkq = nc.declare_dram_parameter("wkq", [128, PD, P], U8, isOutput=False)
    wvq = nc.declare_dram_parameter("wvq", [128, PD, P], U8, isOutput=False)
    wqq = nc.declare_dram_parameter("wqq", [128, PP, C], U8, isOutput=False)
    woq = nc.declare_dram_parameter("woq", [128, PP, C], U8, isOutput=False)
    outq = nc.declare_dram_parameter("outq", [BPC, 128, PC, HW], U8, isOutput=True)

    with SplitDrainTileContext(nc) as tc:
        with (
            tc.tile_pool(name="persist", bufs=1) as persist,
            tc.tile_pool(name="ps_a", bufs=2, space="PSUM") as ps_a,
            tc.tile_pool(name="ps_o", bufs=4, space="PSUM") as ps_o,
        ):
            for rep in range(reps):
                # ---- persistent tiles ----
                wk_sb = persist.tile([128, PD, P], F8, tag="wk")
                wv_sb = persist.tile([128, PD, P], F8, tag="wv")
                wqt_sb = persist.tile([128, PP, C], F8, tag="wqt")
                wo_sb = persist.tile([128, PP, C], F8, tag="wo")
                yt_sb = persist.tile([128, PD, NB], F8, tag="yt")
                x_sb = [persist.tile([128, PC, HW], F8, tag=f"x{b}", name=f"x_sb{b}") for b in range(BPC)]
                kt8 = persist.tile([128, PP, NB], F8, tag="kt8")
                vt8 = persist.tile([128, PP, NB], F8, tag="vt8")
                m8 = persist.tile([128, PC, NB], F8, tag="m8")
                vw8 = [persist.tile([128, C], F8, tag=f"vw{b}", name=f"vw8_{b}") for b in range(BPC)]
                a8 = [persist.tile([128, HW], F8, tag=f"a{b}", name=f"a8_{b}") for b in range(BPC)]
                out8 = [persist.tile([128, PC, HW], F8, tag=f"o{b}", name=f"out8_{b}") for b in range(BPC)]
                part = [persist.tile([128, HW // 512], F32, tag=f"pt{b}", name=f"part{b}") for b in range(BPC)]
                zsum = [persist.tile([128, 1], F32, tag=f"z{b}", name=f"zsum{b}") for b in range(BPC)]
                rz = [persist.tile([128, 1], F32, tag=f"rz{b}", name=f"rz{b}") for b in range(BPC)]
                bias_sb = persist.tile([128, 1], F32, tag="bias")
                warm = persist.tile([128, 1], F32, tag="warm")

                # ---- loads (SP/HWDGE) ----
                def load_x(b, s):
                    nc.sync.dma_start(
                        out=u8(x_sb[b][:, :, s * 1024 : (s + 1) * 1024]),
                        in_=xq[b, :, :, s * 1024 : (s + 1) * 1024],
                    )

                nc.sync.dma_start(out=u8(wk_sb), in_=wkq[:, :, :])
                nc.sync.dma_start(out=u8(yt_sb), in_=yq[:, :, :, :])
                nc.sync.dma_start(out=u8(wqt_sb), in_=wqq[:, :, :])
                for s in range(NJJ):
                    load_x(0, s)

                nc.gpsimd.memset(bias_sb, -LN4)
                # pre-warm the Exp activation table during the weight DMAs
                nc.scalar.activation(out=warm, in_=bias_sb, func=EXP, scale=0.0,
                                     bias=bias_sb)

                # ---- preamble: kt8, vt8, m8 (PE DoubleRow + Act copies) ----
                for pc in range(PP):
                    ps = ps_a.tile([128, 512], F32, tag="pa")
                    for i in range(PD // 2):
                        nc.tensor.matmul(
                            ps[:, :NB],
                            wk_sb[:, 2 * i : 2 * i + 2, pc * 128 : (pc + 1) * 128],
                            yt_sb[:, 2 * i : 2 * i + 2, :],
                            start=(i == 0),
                            stop=(i == PD // 2 - 1),
                            perf_mode=DR,
                        )
                    nc.vector.tensor_scalar_mul(kt8[:, pc, :], ps[:, :NB], 2.0**-7)
                for cc in range(PC):
                    ps = ps_a.tile([128, 512], F32, tag="pa")
                    for i in range(PP // 2):
                        nc.tensor.matmul(
                            ps[:, :NB],
                            wqt_sb[:, 2 * i : 2 * i + 2, cc * 128 : (cc + 1) * 128],
                            kt8[:, 2 * i : 2 * i + 2, :],
                            start=(i == 0),
                            stop=(i == PP // 2 - 1),
                            perf_mode=DR,
                        )
                    nc.vector.tensor_scalar_mul(m8[:, cc, :], ps[:, :NB], 2.0**-8)

                # ---- main: scores/exp b0, then out-b0 interleaved with
                # scores/exp b1 (keeps DVE/Act copy work overlapped with
                # the b1 softmax), then out-b1. ----
                NJ = HW // 512

                def scores_chunk(b, j):
                    ps = ps_a.tile([128, 512], F32, tag="pa", name=f"ps_s{b}_{j}")
                    col = j * 512
                    for i in range(PC // 2):
                        nc.tensor.matmul(
                            ps,
                            m8[:, 2 * i : 2 * i + 2, b * 128 : (b + 1) * 128],
                            x_sb[b][:, 2 * i : 2 * i + 2, col : col + 512],
                            start=(i == 0),
                            stop=(i == PC // 2 - 1),
                            perf_mode=DR,
                        )
                    nc.scalar.activation(
                        out=a8[b][:, col : col + 512],
                        in_=ps,
                        func=EXP,
                        scale=2.0**-10,
                        bias=bias_sb,
                        accum_out=part[b][:, j : j + 1],
                    )

                def vw_mm(b):
                    psv = ps_a.tile([128, 512], F32, tag="pa", name=f"ps_vw{b}")
                    for i in range(PP // 2):
                        nc.tensor.matmul(
                            psv,
                            vt8[:, 2 * i : 2 * i + 2, b * 128 : (b + 1) * 128],
                            wo_sb[:, 2 * i : 2 * i + 2, :],
                            start=(i == 0),
                            stop=(i == PP // 2 - 1),
                            perf_mode=DR,
                        )
                    return psv

                def norm(b, psv):
                    nc.vector.reduce_sum(out=zsum[b], in_=part[b], axis=AX)
                    nc.vector.reciprocal(out=rz[b], in_=zsum[b])
                    nc.vector.tensor_scalar_mul(vw8[b], psv, rz[b])

                cp_k = [0]

                def out_chunk(b, j, rot):
                    col = j * 512
                    for cc in range(PC):
                        ps = ps_o.tile([128, 512], F32, tag="po", name=f"ps_o{b}_{j}_{cc}")
                        nc.tensor.matmul(
                            ps,
                            vw8[b][:, cc * 128 : (cc + 1) * 128],
                            a8[b][:, col : col + 512],
                            start=True,
                            stop=True,
                        )
                        eng = rot[cp_k[0] % len(rot)]
                        cp_k[0] += 1
                        dst = out8[b][:, cc, col : col + 512]
                        if eng is nc.scalar:
                            eng.copy(dst, ps)
                        else:
                            eng.tensor_copy(dst, ps)

                def store(b, jj):
                    nc.gpsimd.dma_start(
                        out=outq[b, :, :, jj * 1024 : (jj + 1) * 1024],
                        in_=u8(out8[b][:, :, jj * 1024 : (jj + 1) * 1024]),
                    )

                ROT = [nc.vector, nc.scalar]
                for j in range(NJ):
                    scores_chunk(0, j)
                # deferred loads: emitted late so earlier consumers'
                # conservative SP-queue waits don't cover them
                nc.sync.dma_start(out=u8(wv_sb), in_=wvq[:, :, :])
                nc.sync.dma_start(out=u8(wo_sb), in_=woq[:, :, :])
                load_x(1, 0)
                for pc in range(PP):
                    ps = ps_a.tile([128, 512], F32, tag="pa", name=f"ps_vt{pc}")
                    for i in range(PD // 2):
                        nc.tensor.matmul(
                            ps[:, :NB],
                            wv_sb[:, 2 * i : 2 * i + 2, pc * 128 : (pc + 1) * 128],
                            yt_sb[:, 2 * i : 2 * i + 2, :],
                            start=(i == 0),
                            stop=(i == PD // 2 - 1),
                            perf_mode=DR,
                        )
                    nc.vector.tensor_scalar_mul(vt8[:, pc, :], ps[:, :NB], 2.0**-7)
                psv0 = vw_mm(0)
                norm(0, psv0)
                for j in range(NJ):
                    if j in (2, 4, 6):
                        load_x(1, j // 2)
                    out_chunk(0, j, ROT)
                    scores_chunk(1, j)
                    if j % 2 == 1:
                        store(0, j // 2)
                psv1 = vw_mm(1)
                norm(1, psv1)
                for j in range(NJ):
                    out_chunk(1, j, ROT)
                    if j % 2 == 1:
                        store(1, j // 2)
    if split_waits:
        split_multi_waits(nc)
    return nc


def shard_inputs(x, y, Wq, Wk, Wv, Wout):
    """Host-side packing: fp8-quantize with power-of-two scales and
    lay out so each device load is one contiguous DMA."""
    x = np.asarray(x, np.float32)
    y = np.asarray(y, np.float32)

    def q8(a, scale):
        return (a * np.float32(scale)).astype(NPF8)

    # x' : [B, p, cc, HW]
    xq_full = np.ascontiguousarray(
        q8(x.reshape(B, PC, 128, HW), 2.0**3).transpose(0, 2, 1, 3)
    )
    # y' : [p, dc, B, n]
    yq_full = np.ascontiguousarray(
        q8(np.asarray(y, np.float32), 2.0**3)
        .reshape(B, N_COND, PD, 128)
        .transpose(3, 2, 0, 1)
    )
    wkq = np.ascontiguousarray(
        q8(np.asarray(Wk, np.float32), 2.0**7).reshape(PD, 128, P).transpose(1, 0, 2)
    )
    wvq = np.ascontiguousarray(
        q8(np.asarray(Wv, np.float32), 2.0**7).reshape(PD, 128, P).transpose(1, 0, 2)
    )
    wqs = np.asarray(Wq, np.float32) * np.float32(2.0**12 / np.sqrt(C))
    wqq = np.ascontiguousarray(
        wqs.T.astype(NPF8).reshape(PP, 128, C).transpose(1, 0, 2)
    )
    woq = np.ascontiguousarray(
        q8(np.asarray(Wout, np.float32), 2.0**6).reshape(PP, 128, C).transpose(1, 0, 2)
    )

    in_maps = []
    for core in range(N_CORES):
        b0 = core * BPC
        in_maps.append(
            {
                "xq": xq_full[b0 : b0 + BPC].view(np.uint8),
                "yq": np.ascontiguousarray(yq_full[:, :, b0 : b0 + BPC, :]).view(
                    np.uint8
                ),
                "wkq": wkq.view(np.uint8),
                "wvq": wvq.view(np.uint8),
                "wqq": wqq.view(np.uint8),
                "woq": woq.view(np.uint8),
            }
        )
    return in_maps


def kernel(x, y, Wq, Wk, Wv, Wout):
    from concourse.bass_utils import run_bass_kernel_spmd

    nc = build_nc(reps=1)
    in_maps = shard_inputs(x, y, Wq, Wk, Wv, Wout)
    res = run_bass_kernel_spmd(nc, in_maps, list(range(N_CORES)))
    attn = np.empty((B, 128, PC, HW), dtype=np.float32)
    for core in range(N_CORES):
        b0 = core * BPC
        attn[b0 : b0 + BPC] = (
            np.asarray(res.results[core]["outq"]).view(NPF8).astype(np.float32)
        )
    out = np.asarray(x, np.float32).reshape(B, C, HW) + attn.transpose(
        0, 2, 1, 3
    ).reshape(B, C, HW) * np.float32(2.0**-9)
    return out.reshape(B, C, H, W)



# revision 2
# speedup vs baseline: 3.0782x; 3.0782x over previous
"""Trainium2 Bass kernel for nn_CrossAttentionBlock — all-fp8 pipeline, v2.

Reference computation (per batch b of 16):
    q  = einsum('chw,cp->hwp', x[b], Wq)      # (HW=4096, P=512)
    kt = einsum('nd,dp->pn',  y[b], Wk)       # (P, N=128)
    v  = y[b] @ Wv                            # (N, P)
    s  = (q @ kt) / sqrt(C)                   # (HW, N)
    a  = softmax(s, axis=HW)                  # over the SPATIAL axis
    o  = (a @ v) @ Wout                       # (HW, C)
    out = x + o.T.reshape(C, H, W)

Sharding: pure data-parallel over batch, 2 batches per core, no
collectives.

v2 structure (changes vs v1 noted):
  * Host weight fold:  GV = Wv @ Wout  (D, C).  On device
        vw_psum = y' @ GV'            (N, C) = VW * 2^9
    replacing the two-stage vt/VW pipeline (saves 12 PE matmuls and
    0.25 MB of weight DMA per core; single fp8 quantization of the
    folded weight is also slightly more accurate).
  * M  = (Wq/sqrt(C)) @ (Wk^T y^T)   (C, N)  once per batch (as v1):
        kt8 = fp8(Wk'^T y' * 2^-7); m8 = fp8(Wq'^T kt8 * 2^-8)
  * scores s^T = m8^T x' into paired [128,1024] PSUM tiles (2 banks);
    ONE exp activation per 1024 columns (halves Act instruction count),
    accum_out collects the softmax-Z partials.
  * out_psum = vw8^T a8 into paired [128,1024] PSUM tiles; ONE
    PSUM->SBUF fp8 copy per 1024 columns, rotated across Pool/DVE/Act
    (3-way split keeps all three engines below the PE's busy time).
  * Residual add + dequant on host:  out = x + out8 * 2^-9.

Scale bookkeeping (all powers of two; softmax is invariant to the
exp-output scale since Z comes from the same activation's accumulator):
    x' = x*2^3   y' = y*2^3   Wk' = Wk*2^7   GV' = GV*2^6
    Wq' = (Wq/sqrt(C))*2^12
    kt_psum = Wk'^T y'     = kt*2^10 ; kt8 = kt_psum*2^-7
    M_psum  = Wq'^T kt8    = M *2^15 ; m8  = M_psum *2^-8
    s_psum  = m8^T x'      = s *2^10 ; a8  = exp(s_psum*2^-10 - ln4)
    vw_psum = y' GV'       = VW*2^9  ; vw8 = vw_psum * (1/Z')
    out_psum = vw8^T a8    = attnout*2^9 ; host adds x + out8*2^-9
"""

import sys

sys.path.insert(0, "/opt/trn_rl_repo")

import numpy as np
import ml_dtypes

import concourse.bass as bass
import concourse.mybir as mybir
import concourse.tile as tile
from concourse.vector_clock import ScopedClock

B, C, H, W = 16, 512, 64, 64
HW = H * W
N_COND, D_COND, P = 128, 1024, 512
N_CORES = 8
BPC = B // N_CORES  # batches per core

F32 = mybir.dt.float32
F8 = mybir.dt.float8e4
U8 = mybir.dt.uint8
NPF8 = ml_dtypes.float8_e4m3
AX = mybir.AxisListType.X
EXP = mybir.ActivationFunctionType.Exp
DR = mybir.MatmulPerfMode.DoubleRow

PC = C // 128       # 4 chunks over C
PP = P // 128       # 4 chunks over P
PD = D_COND // 128  # 8 chunks over D
NG = HW // 1024     # 4 column groups of 1024
NB = BPC * 128      # 256: batch-concatenated N columns

LN4 = float(np.log(4.0))


class SplitDrainTileContext(tile.TileContext):
    """This walrus build accepts only one sem wait per CTRL/drain
    instruction; Tile's tail drain waits on the whole global clock.
    Split the waits across a chain of drains on SP."""

    MAX_WAITS = 1

    def _drain_and_barrier(self, tick_clock, wait_clock):
        drain_inst = self.nc.sync.drain()
        wait_clock.add_sem_waits(
            drain_inst.ins, ScopedClock({None: tick_clock.global_clock})
        )
        si = drain_inst.ins.sync_info
        if si is not None and si.on_wait and len(si.on_wait) > self.MAX_WAITS:
            waits = list(si.on_wait)
            drain_inst.ins.sync_info = mybir.SyncInfo(
                on_wait=waits[: self.MAX_WAITS],
                on_update=list(si.on_update or []),
            )
            for i in range(self.MAX_WAITS, len(waits), self.MAX_WAITS):
                extra = self.nc.sync.drain()
                extra.ins.sync_info = mybir.SyncInfo(
                    on_wait=waits[i : i + self.MAX_WAITS], on_update=[]
                )
        self.nc.all_engine_barrier()
        assert self.sems is not None
        popped = self.nc._tile_sem_poison_stack.pop()
        assert popped is self._sem_poison
        self.nc.clear_and_free_semaphores(list(self.sems.allocated().values()))
        self.nc.all_engine_barrier()


def split_multi_waits(nc):
    """This walrus build's codegen accepts at most ONE sem wait per
    instruction (any struct type). Split extra waits onto same-engine
    NoOps inserted immediately before the instruction."""
    ctr = [0]
    for fn in nc.m.functions:
        for bb in fn.blocks:
            insts = bb.instructions
            new = []
            changed = False
            for inst in insts:
                si = inst.sync_info
                if si is not None and si.on_wait and len(si.on_wait) > 1:
                    waits = list(si.on_wait)
                    for w in waits[:-1]:
                        nop = mybir.InstNoOp(
                            name=f"I-wsplit-{ctr[0]}", ins=[], outs=[]
                        )
                        ctr[0] += 1
                        nop.engine = inst.engine
                        nop.sync_info = mybir.SyncInfo(on_wait=[w], on_update=[])
                        new.append(nop)
                    inst.sync_info = mybir.SyncInfo(
                        on_wait=[waits[-1]], on_update=list(si.on_update or [])
                    )
                    changed = True
                new.append(inst)
            if changed:
                bb.instructions = new


def u8(ap):
    """View an fp8 AP as uint8 so DMA endpoints match the u8 DRAM
    tensors (avoids fp8 dtypes in the jax/PJRT boundary)."""
    return ap.bitcast(U8)


def build_nc(reps: int = 1, split_waits: bool = True) -> bass.Bass:
    nc = bass.Bass()

    # All DRAM tensors are uint8 views of fp8 payloads, host-packed so
    # every load is one contiguous-per-partition DMA.
    xq = nc.declare_dram_parameter("xq", [BPC, 128, PC, HW], U8, isOutput=False)
    yq = nc.declare_dram_parameter("yq", [128, PD, BPC, 128], U8, isOutput=False)
    wkq = nc.declare_dram_parameter("wkq", [128, PD, P], U8, isOutput=False)
    wqq = nc.declare_dram_parameter("wqq", [128, PP, C], U8, isOutput=False)
    gvq = nc.declare_dram_parameter("gvq", [128, PD, C], U8, isOutput=False)
    outq = nc.declare_dram_parameter("outq", [BPC, 128, PC, HW], U8, isOutput=True)

    with SplitDrainTileContext(nc) as tc:
        with (
            tc.tile_pool(name="persist", bufs=1) as persist,
            tc.tile_pool(name="ps_a", bufs=2, space="PSUM") as ps_a,
            tc.tile_pool(name="ps_o", bufs=2, space="PSUM") as ps_o,
        ):
            for rep in range(reps):
                # ---- persistent tiles ----
                wk_sb = persist.tile([128, PD, P], F8, tag="wk")
                wqt_sb = persist.tile([128, PP, C], F8, tag="wqt")
                gv_sb = persist.tile([128, PD, C], F8, tag="gv")
                yt_sb = persist.tile([128, PD, NB], F8, tag="yt")
                x_sb = [persist.tile([128, PC, HW], F8, tag=f"x{b}", name=f"x_sb{b}") for b in range(BPC)]
                kt8 = persist.tile([128, PP, NB], F8, tag="kt8")
                m8 = persist.tile([128, PC, NB], F8, tag="m8")
                vw8 = [persist.tile([128, C], F8, tag=f"vw{b}", name=f"vw8_{b}") for b in range(BPC)]
                a8 = [persist.tile([128, HW], F8, tag=f"a{b}", name=f"a8_{b}") for b in range(BPC)]
                out8 = [persist.tile([128, PC, HW], F8, tag=f"o{b}", name=f"out8_{b}") for b in range(BPC)]
                part = [persist.tile([128, NG], F32, tag=f"pt{b}", name=f"part{b}") for b in range(BPC)]
                zsum = [persist.tile([128, 1], F32, tag=f"z{b}", name=f"zsum{b}") for b in range(BPC)]
                rz = [persist.tile([128, 1], F32, tag=f"rz{b}", name=f"rz{b}") for b in range(BPC)]
                bias_sb = persist.tile([128, 1], F32, tag="bias")
                warm = persist.tile([128, 1], F32, tag="warm")

                # ---- loads (SP/HWDGE) ----
                def load_x(b, s):
                    # halves: s in (0, 1), 2048 cols each (1 MB)
                    nc.sync.dma_start(
                        out=u8(x_sb[b][:, :, s * 2048 : (s + 1) * 2048]),
                        in_=xq[b, :, :, s * 2048 : (s + 1) * 2048],
                    )

                nc.sync.dma_start(out=u8(wk_sb), in_=wkq[:, :, :])
                nc.sync.dma_start(out=u8(yt_sb), in_=yq[:, :, :, :])
                nc.sync.dma_start(out=u8(wqt_sb), in_=wqq[:, :, :])
                load_x(0, 0)
                load_x(0, 1)

                nc.gpsimd.memset(bias_sb, -LN4)
                # pre-warm the Exp activation table during the weight DMAs
                nc.scalar.activation(out=warm, in_=bias_sb, func=EXP, scale=0.0,
                                     bias=bias_sb)

                # ---- preamble: kt8, m8 (PE DoubleRow + DVE scale-copies) --
                for pc in range(PP):
                    ps = ps_a.tile([128, 1024], F32, tag="pa", name=f"ps_kt{pc}")
                    for i in range(PD // 2):
                        nc.tensor.matmul(
                            ps[:, :NB],
                            wk_sb[:, 2 * i : 2 * i + 2, pc * 128 : (pc + 1) * 128],
                            yt_sb[:, 2 * i : 2 * i + 2, :],
                            start=(i == 0),
                            stop=(i == PD // 2 - 1),
                            perf_mode=DR,
                        )
                    nc.vector.tensor_scalar_mul(kt8[:, pc, :], ps[:, :NB], 2.0**-7)
                for cc in range(PC):
                    ps = ps_a.tile([128, 1024], F32, tag="pa", name=f"ps_m{cc}")
                    for i in range(PP // 2):
                        nc.tensor.matmul(
                            ps[:, :NB],
                            wqt_sb[:, 2 * i : 2 * i + 2, cc * 128 : (cc + 1) * 128],
                            kt8[:, 2 * i : 2 * i + 2, :],
                            start=(i == 0),
                            stop=(i == PP // 2 - 1),
                            perf_mode=DR,
                        )
                    nc.vector.tensor_scalar_mul(m8[:, cc, :], ps[:, :NB], 2.0**-8)

                # ---- per-batch stages ----
                def scores_group(b, g):
                    """1024 columns of scores -> one exp activation."""
                    ps = ps_a.tile([128, 1024], F32, tag="pa", name=f"ps_s{b}_{g}")
                    for half in range(2):
                        col = g * 1024 + half * 512
                        for i in range(PC // 2):
                            nc.tensor.matmul(
                                ps[:, half * 512 : (half + 1) * 512],
                                m8[:, 2 * i : 2 * i + 2, b * 128 : (b + 1) * 128],
                                x_sb[b][:, 2 * i : 2 * i + 2, col : col + 512],
                                start=(i == 0),
                                stop=(i == PC // 2 - 1),
                                perf_mode=DR,
                            )
                    nc.scalar.activation(
                        out=a8[b][:, g * 1024 : (g + 1) * 1024],
                        in_=ps,
                        func=EXP,
                        scale=2.0**-10,
                        bias=bias_sb,
                        accum_out=part[b][:, g : g + 1],
                    )

                def vw_mm(b):
                    """vw_psum = y_b' @ GV' = VW*2^9   (N=128, C=512)."""
                    psv = ps_a.tile([128, 1024], F32, tag="pa", name=f"ps_vw{b}")
                    for i in range(PD // 2):
                        nc.tensor.matmul(
                            psv[:, :512],
                            yt_sb[:, 2 * i : 2 * i + 2, b * 128 : (b + 1) * 128],
                            gv_sb[:, 2 * i : 2 * i + 2, :],
                            start=(i == 0),
                            stop=(i == PD // 2 - 1),
                            perf_mode=DR,
                        )
                    return psv

                def norm(b, psv):
                    nc.vector.reduce_sum(out=zsum[b], in_=part[b], axis=AX)
                    nc.vector.reciprocal(out=rz[b], in_=zsum[b])
                    nc.vector.tensor_scalar_mul(vw8[b], psv[:, :512], rz[b])

                # copy rotation: Pool is cheapest, Act carries the exps.
                COPY_ROT = ["P", "D", "P", "D", "P", "D", "A", "P"]
                cp_k = [0]

                def out_group(b, g):
                    """1024 columns of attnout^T for all 4 C-chunks."""
                    col = g * 1024
                    for cc in range(PC):
                        ps = ps_o.tile([128, 1024], F32, tag="po",
                                       name=f"ps_o{b}_{g}_{cc}")
                        for half in range(2):
                            nc.tensor.matmul(
                                ps[:, half * 512 : (half + 1) * 512],
                                vw8[b][:, cc * 128 : (cc + 1) * 128],
                                a8[b][:, col + half * 512 : col + (half + 1) * 512],
                                start=True,
                                stop=True,
                            )
                        eng = COPY_ROT[cp_k[0] % len(COPY_ROT)]
                        cp_k[0] += 1
                        dst = out8[b][:, cc, col : col + 1024]
                        if eng == "A":
                            nc.scalar.copy(dst, ps)
                        elif eng == "D":
                            nc.vector.tensor_copy(dst, ps)
                        else:
                            nc.gpsimd.tensor_copy(dst, ps)

                def store(b, g):
                    nc.gpsimd.dma_start(
                        out=outq[b, :, :, g * 1024 : (g + 1) * 1024],
                        in_=u8(out8[b][:, :, g * 1024 : (g + 1) * 1024]),
                    )

                # ---- schedule ----
                for g in range(NG):
                    scores_group(0, g)
                # deferred loads: emitted late so earlier consumers'
                # conservative SP-queue waits don't cover them
                nc.sync.dma_start(out=u8(gv_sb), in_=gvq[:, :, :])
                load_x(1, 0)
                load_x(1, 1)
                psv0 = vw_mm(0)
                norm(0, psv0)
                for g in range(NG):
                    out_group(0, g)
                    scores_group(1, g)
                    store(0, g)
                psv1 = vw_mm(1)
                norm(1, psv1)
                for g in range(NG):
                    out_group(1, g)
                    store(1, g)
    if split_waits:
        split_multi_waits(nc)
    return nc


def shard_inputs(x, y, Wq, Wk, Wv, Wout):
    """Host-side packing: fp8-quantize with power-of-two scales, fold
    GV = Wv @ Wout, and lay out so each device load is one
    contiguous-per-partition DMA."""
    x = np.asarray(x, np.float32)
    y = np.asarray(y, np.float32)

    def q8(a, scale):
        return (a * np.float32(scale)).astype(NPF8)

    # x' : [B, p, cc, HW]
    xq_full = np.ascontiguousarray(
        q8(x.reshape(B, PC, 128, HW), 2.0**3).transpose(0, 2, 1, 3)
    )
    # y' : [p, dc, B, n]
    yq_full = np.ascontiguousarray(
        q8(y, 2.0**3).reshape(B, N_COND, PD, 128).transpose(3, 2, 0, 1)
    )
    wkq = np.ascontiguousarray(
        q8(np.asarray(Wk, np.float32), 2.0**7).reshape(PD, 128, P).transpose(1, 0, 2)
    )
    wqs = np.asarray(Wq, np.float32) * np.float32(2.0**12 / np.sqrt(C))
    wqq = np.ascontiguousarray(
        wqs.T.astype(NPF8).reshape(PP, 128, C).transpose(1, 0, 2)
    )
    gv = np.asarray(Wv, np.float32) @ np.asarray(Wout, np.float32)  # (D, C)
    gvq = np.ascontiguousarray(
        q8(gv, 2.0**6).reshape(PD, 128, C).transpose(1, 0, 2)
    )

    in_maps = []
    for core in range(N_CORES):
        b0 = core * BPC
        in_maps.append(
            {
                "xq": xq_full[b0 : b0 + BPC].view(np.uint8),
                "yq": np.ascontiguousarray(yq_full[:, :, b0 : b0 + BPC, :]).view(
                    np.uint8
                ),
                "wkq": wkq.view(np.uint8),
                "wqq": wqq.view(np.uint8),
                "gvq": gvq.view(np.uint8),
            }
        )
    return in_maps


def kernel(x, y, Wq, Wk, Wv, Wout):
    from concourse.bass_utils import run_bass_kernel_spmd

    nc = build_nc(reps=1)
    in_maps = shard_inputs(x, y, Wq, Wk, Wv, Wout)
    res = run_bass_kernel_spmd(nc, in_maps, list(range(N_CORES)))
    attn = np.empty((B, 128, PC, HW), dtype=np.float32)
    for core in range(N_CORES):
        b0 = core * BPC
        attn[b0 : b0 + BPC] = (
            np.asarray(res.results[core]["outq"]).view(NPF8).astype(np.float32)
        )
    out = np.asarray(x, np.float32).reshape(B, C, HW) + attn.transpose(
        0, 2, 1, 3
    ).reshape(B, C, HW) * np.float32(2.0**-9)
    return out.reshape(B, C, H, W)
